# revision 31
# baseline (speedup 1.0000x reference)
"""Trainium2 Bass kernel for the skip-LSTM autoencoder.

Strategy (8 NeuronCores, zero collectives):
  - Every core runs the full-batch (B=64) encoder + decoder recurrences
    replicated (they are latency-bound; replication costs nothing extra).
  - The 16384x16384 dense layer is column-sharded: core c computes output
    columns [2048c, 2048(c+1)).  The weight shard is pre-transposed on the
    host to (K=16384, J=2048) bf16 so k-tiles stream as perfectly
    contiguous [128, 2048] DMA blocks.
  - All state lives transposed: [U=128 partitions, B=64 free].  Each
    decoder step's hidden tile h_t^T is immediately the dense matmul
    stationary (lhsT); the dense output accumulates in PSUM across all
    128 timesteps and is evacuated once.

Structure facts exploited from the reference recurrence:
  - The i/f/o gate chains are self-contained (gate_t depends only on
    gate_{t-1} and x_t); only the candidate path consumes c_{t-1}.
  - h_t is consumed by nothing except the +16-step skip connection and
    the dense layer, so h is computed in 16-wide batches off the
    critical path.
"""

import sys
from contextlib import ExitStack

sys.path.insert(0, "/opt/trn_rl_repo")

import numpy as np
import ml_dtypes

import concourse.bass as bass
import concourse.mybir as mybir
import concourse.tile as tile
from concourse import bacc
from concourse.bass_utils import run_bass_kernel_spmd

BF16 = mybir.dt.bfloat16
F32 = mybir.dt.float32
AF = mybir.ActivationFunctionType
ALU = mybir.AluOpType

U = 128      # units
B = 64       # batch
T = 128      # sequence length
M = 128      # input feature dim
SKIP = 16
NCORES = 8
JSH = (T * M) // NCORES  # 2048 output columns per core

nbf = ml_dtypes.bfloat16

# module-level cache: (key) -> (nc, meta)
_GRAPH_CACHE = {}


def _lstm_phase(nc, tc, pools, cfg, *, is_enc, k_tiles, rk_tiles, k2_tile,
                b2_col, bias_row, ones_row, s0, xc_tile, ident, x1t_tile,
                c_hist, h_hist, ifo_ring, enc_h_chain, dense_fn):
    """Emit one skip-LSTM unroll (128 steps).

    is_enc: encoder (per-step x matmuls from x1t_tile, h only for the
            t%16==15 chain) vs decoder (constant xc via identity matmul,
            h for every step via 16-wide batched tails + dense_fn).
    """
    psum_g = pools["psum_g"]
    tmp = pools["tmp"]
    tmpb = pools["tmpb"]
    gp_ps = pools["skip_ps"]

    for t in range(T):
        # ---- gate pre-activations: psum [128, 256] = [i | f | o | cand]
        # PSUM gotcha: start=True resets has_written for the WHOLE bank, so
        # exactly one full-width start matmul must come first.
        ps = psum_g.tile([U, 4 * B], F32, tag="gates_ps")
        if is_enc:
            # full-width start: bias broadcast via K=4 gate-select matmul
            nc.tensor.matmul(ps, bias_row, ones_row, start=True, stop=False)
            # x-term: 4 matmuls kernel_g^T @ x_t
            for g in range(4):
                gsl = ps[:, g * B:(g + 1) * B]
                nc.tensor.matmul(gsl, k_tiles[g], x1t_tile[:, t, :],
                                 start=False, stop=(t == 0),
                                 skip_group_check=True)
        else:
            # constant x-term (incl. bias): one identity matmul, N=256
            nc.tensor.matmul(ps, ident, xc_tile, start=True, stop=(t == 0))
        if t > 0:
            prev = ifo_ring[:, (t - 1) % 32, :]
            nc.tensor.matmul(ps[:, 0:B], rk_tiles[0], prev[:, 0:B],
                             start=False, stop=True, skip_group_check=True)
            nc.tensor.matmul(ps[:, B:2 * B], rk_tiles[1], prev[:, B:2 * B],
                             start=False, stop=True, skip_group_check=True)
            nc.tensor.matmul(ps[:, 2 * B:3 * B], rk_tiles[3], prev[:, 2 * B:3 * B],
                             start=False, stop=True, skip_group_check=True)
            nc.tensor.matmul(ps[:, 3 * B:4 * B], rk_tiles[2], c_hist[:, t - 1, :],
                             start=False, stop=True, skip_group_check=True)

        # ---- activations: one sigmoid over [i,f,o], one tanh for cand
        if is_enc and t == 1 and cfg.get("debug", False):
            ps1c = pools["tmpb"].tile([U, 4 * B], F32, tag="dbg_ps1c",
                                      name="ps1c")
            nc.vector.tensor_copy(ps1c, ps)
            nc.sync.dma_start(cfg["d_dbg_ps1"], ps1c)
            nc.sync.dma_start(cfg["d_dbg_ifo0"], ifo_ring[:, 0, :])
        ifo = ifo_ring[:, t % 32, :]
        nc.scalar.activation(ifo, ps[:, 0:3 * B], AF.Sigmoid)
        u_t = tmp.tile([U, B], BF16, tag="u")
        nc.scalar.activation(u_t, ps[:, 3 * B:4 * B], AF.Tanh)

        # ---- cell update: c_t = f*c_{t-1} + i*u   (bf16 state)
        if t == 0:
            nc.vector.tensor_tensor(c_hist[:, 0, :], ifo[:, 0:B], u_t,
                                    ALU.mult)
        else:
            m1 = tmp.tile([U, B], BF16, tag="m1")
            nc.vector.tensor_tensor(m1, ifo[:, 0:B], u_t, ALU.mult)
            fb = tmp.tile([U, B], BF16, tag="fb")
            nc.gpsimd.tensor_tensor(fb, ifo[:, B:2 * B], c_hist[:, t - 1, :],
                                    ALU.mult)
            nc.vector.tensor_tensor(c_hist[:, t, :], m1, fb, ALU.add)

        # ---- h tail
        if is_enc:
            if t % SKIP == SKIP - 1:
                ci = t // SKIP          # chain index 0..7
                h_prev = enc_h_chain[ci]  # holds h_{t-16} (ci==0: zeros)
                sps = gp_ps.tile([U, SKIP * B], F32, tag="skip_ps")
                nc.tensor.matmul(sps[:, 0:B], k2_tile, h_prev,
                                 start=True, stop=True, skip_group_check=True)
                skp = tmp.tile([U, B], BF16, tag="skp")
                nc.scalar.activation(skp, sps[:, 0:B], AF.Sigmoid, bias=b2_col)
                tc_t = tmp.tile([U, B], BF16, tag="tc")
                nc.scalar.activation(tc_t, c_hist[:, t, :], AF.Tanh)
                hc = tmp.tile([U, B], BF16, tag="hc")
                nc.vector.tensor_tensor(hc, ifo[:, 2 * B:3 * B], tc_t, ALU.mult)
                skp2 = tmp.tile([U, B], BF16, tag="skp2")
                nc.gpsimd.tensor_scalar_mul(skp2, skp, 1.0 - s0)
                nc.vector.scalar_tensor_tensor(
                    enc_h_chain[ci + 1], hc, s0, skp2, ALU.mult, ALU.add)
        else:
            if t % SKIP == SKIP - 1:
                t0 = t - (SKIP - 1)
                W = SKIP * B  # 1024
                # skip pre-act for steps t0..t0+15: inputs h_{t0-16..t0-1}
                # = h_hist slices [t0 .. t0+16)  (h_hist has 16 zero slices
                # in front).  Two N=512 matmuls: PSUM bank limit.
                sps = gp_ps.tile([U, W], F32, tag="skip_ps")
                nc.tensor.matmul(sps[:, 0:W // 2], k2_tile,
                                 h_hist[:, t0:t0 + SKIP // 2, :],
                                 start=True, stop=True, skip_group_check=True)
                nc.tensor.matmul(sps[:, W // 2:W], k2_tile,
                                 h_hist[:, t0 + SKIP // 2:t0 + SKIP, :],
                                 start=True, stop=True, skip_group_check=True)
                skp = tmpb.tile([U, W], BF16, tag="skp_b")
                nc.scalar.activation(skp, sps, AF.Sigmoid, bias=b2_col)
                tc_b = tmpb.tile([U, W], BF16, tag="tc_b")
                nc.scalar.activation(
                    tc_b,
                    c_hist[:, t0:t0 + SKIP, :].rearrange("p s b -> p (s b)"),
                    AF.Tanh)
                hc = tmpb.tile([U, W], BF16, tag="hc_b")
                nc.vector.tensor_tensor(
                    hc.rearrange("p (s b) -> p s b", s=SKIP),
                    ifo_ring[:, t0 % 32:t0 % 32 + SKIP, 2 * B:3 * B],
                    tc_b.rearrange("p (s b) -> p s b", s=SKIP), ALU.mult)
                skp2 = tmpb.tile([U, W], BF16, tag="skp2_b")
                nc.gpsimd.tensor_scalar_mul(skp2, skp, 1.0 - s0)
                nc.vector.scalar_tensor_tensor(
                    h_hist[:, SKIP + t0:SKIP + t0 + SKIP, :].rearrange(
                        "p s b -> p (s b)"),
                    hc, s0, skp2, ALU.mult, ALU.add)
                # dense matmuls for this group of timesteps
                dense_fn(t0)


def build_graph(cfg):
    """Build the SPMD graph (identical on all cores)."""
    nc = bacc.Bacc("TRN2", target_bir_lowering=False, debug=False,
                   num_devices=NCORES)

    # ---------------- DRAM parameters ----------------
    d_x1t = nc.dram_tensor("x1t", [M, T, B], BF16, kind="ExternalInput").ap()
    d_enc_k = nc.dram_tensor("enc_k", [M, 4 * U], BF16, kind="ExternalInput").ap()
    d_enc_rk = nc.dram_tensor("enc_rk", [U, 4 * U], BF16, kind="ExternalInput").ap()
    d_enc_k2 = nc.dram_tensor("enc_k2", [U, U], BF16, kind="ExternalInput").ap()
    d_dec_k = nc.dram_tensor("dec_k", [U, 4 * U], BF16, kind="ExternalInput").ap()
    d_dec_rk = nc.dram_tensor("dec_rk", [U, 4 * U], BF16, kind="ExternalInput").ap()
    d_dec_k2 = nc.dram_tensor("dec_k2", [U, U], BF16, kind="ExternalInput").ap()
    d_enc_b4 = nc.dram_tensor("enc_b4", [4, U], F32, kind="ExternalInput").ap()
    d_gsel = nc.dram_tensor("gsel", [4, 4 * B], F32, kind="ExternalInput").ap()
    d_dec_b = nc.dram_tensor("dec_b", [1, 4 * U], F32, kind="ExternalInput").ap()
    d_enc_b2 = nc.dram_tensor("enc_b2", [U, 1], F32, kind="ExternalInput").ap()
    d_dec_b2 = nc.dram_tensor("dec_b2", [U, 1], F32, kind="ExternalInput").ap()
    d_ident = nc.dram_tensor("ident", [U, U], BF16, kind="ExternalInput").ap()
    d_ones = nc.dram_tensor("ones", [1, 512], F32, kind="ExternalInput").ap()
    d_wt = nc.dram_tensor("wt", [T * M, JSH], BF16, kind="ExternalInput").ap()
    d_db = nc.dram_tensor("db", [1, JSH], F32, kind="ExternalInput").ap()
    d_out = nc.dram_tensor("out", [B, JSH], F32, kind="ExternalOutput").ap()
    dbg = cfg.get("debug", False)
    if dbg:
        pass  # debug tensors declared below; APs stashed into cfg
    if dbg:
        d_dbg_re = nc.dram_tensor("dbg_re", [U, B], BF16,
                                  kind="ExternalOutput").ap()
        d_dbg_c = nc.dram_tensor("dbg_c", [U, T, B], BF16,
                                 kind="ExternalOutput").ap()
        d_dbg_h = nc.dram_tensor("dbg_h", [U, SKIP + T, B], BF16,
                                 kind="ExternalOutput").ap()
        d_dbg_xc = nc.dram_tensor("dbg_xc", [U, 4 * B], BF16,
                                  kind="ExternalOutput").ap()
        d_dbg_ce = nc.dram_tensor("dbg_ce", [U, T, B], BF16,
                                  kind="ExternalOutput").ap()
        d_dbg_he = nc.dram_tensor("dbg_he", [U, 8, B], BF16,
                                  kind="ExternalOutput").ap()
        d_dbg_ps1 = nc.dram_tensor("dbg_ps1", [U, 4 * B], F32,
                                   kind="ExternalOutput").ap()
        d_dbg_ifo0 = nc.dram_tensor("dbg_ifo0", [U, 3 * B], BF16,
                                    kind="ExternalOutput").ap()
        cfg["d_dbg_ps1"] = d_dbg_ps1
        cfg["d_dbg_ifo0"] = d_dbg_ifo0

    enc_s0 = cfg["enc_s0"]
    dec_s0 = cfg["dec_s0"]

    with tile.TileContext(nc) as tc, ExitStack() as ctx:
        consts = ctx.enter_context(tc.tile_pool(name="consts", bufs=1))
        hist = ctx.enter_context(tc.tile_pool(name="hist", bufs=1))
        tmp = ctx.enter_context(tc.tile_pool(name="tmp", bufs=4))
        tmpb = ctx.enter_context(tc.tile_pool(name="tmpb", bufs=2))
        wpool = ctx.enter_context(tc.tile_pool(name="wpool", bufs=cfg["w_bufs"]))
        psum_g = ctx.enter_context(tc.tile_pool(name="psum_g", bufs=2, space="PSUM"))
        skip_ps = ctx.enter_context(tc.tile_pool(name="skip_ps", bufs=1, space="PSUM"))
        psum_d = ctx.enter_context(tc.tile_pool(name="psum_d", bufs=1, space="PSUM"))

        pools = {"psum_g": psum_g, "tmp": tmp, "tmpb": tmpb,
                 "skip_ps": skip_ps}

        # ---------------- load constants ----------------
        x1t = consts.tile([M, T, B], BF16)
        nc.sync.dma_start(x1t, d_x1t)

        def load_w(dram, shape, nm):
            t_ = consts.tile(shape, BF16, tag=nm, name=nm)
            nc.sync.dma_start(t_, dram)
            return t_

        enc_k = load_w(d_enc_k, [M, 4 * U], "enc_k_t")
        enc_rk = load_w(d_enc_rk, [U, 4 * U], "enc_rk_t")
        enc_k2 = load_w(d_enc_k2, [U, U], "enc_k2_t")
        dec_k = load_w(d_dec_k, [U, 4 * U], "dec_k_t")
        dec_rk = load_w(d_dec_rk, [U, 4 * U], "dec_rk_t")
        dec_k2 = load_w(d_dec_k2, [U, U], "dec_k2_t")
        ident = load_w(d_ident, [U, U], "ident_t")

        enc_b2 = consts.tile([U, 1], F32)
        nc.sync.dma_start(enc_b2, d_enc_b2)
        dec_b2 = consts.tile([U, 1], F32)
        nc.sync.dma_start(dec_b2, d_dec_b2)
        ones = consts.tile([1, 512], F32)
        nc.sync.dma_start(ones, d_ones)
        enc_b4 = consts.tile([4, U], F32)
        nc.sync.dma_start(enc_b4, d_enc_b4)
        gsel = consts.tile([4, 4 * B], F32)
        nc.sync.dma_start(gsel, d_gsel)
        dec_b = consts.tile([1, 4 * U], F32)
        nc.sync.dma_start(dec_b, d_dec_b)
        db_row = consts.tile([1, JSH], F32)
        nc.sync.dma_start(db_row, d_db)

        # gate column order in psum: [i | f | o | cand]; reference weight
        # column order is [i | f | cand | o].  Map: psum gate 0->ref 0,
        # 1->ref 1, 2->ref 3, 3->ref 2.
        REF = [0, 1, 3, 2]
        enc_k_g = [enc_k[:, REF[g] * U:(REF[g] + 1) * U] for g in range(4)]
        enc_rk_g = [enc_rk[:, gg * U:(gg + 1) * U] for gg in range(4)]
        dec_rk_g = [dec_rk[:, gg * U:(gg + 1) * U] for gg in range(4)]

        # ---------------- state buffers ----------------
        c_hist = hist.tile([U, T, B], BF16)       # cell state history
        h_hist = hist.tile([U, SKIP + T, B], BF16)  # decoder h (16 zero pad)
        ifo_ring = hist.tile([U, 32, 3 * B], BF16)
        nc.vector.memset(h_hist[:, 0:SKIP, :], 0.0)

        zeros_h = consts.tile([U, B], BF16)
        nc.vector.memset(zeros_h, 0.0)
        enc_h_chain = [zeros_h] + [
            hist.tile([U, B], BF16, tag=f"ehc{i}", name=f"ehc{i}")
            for i in range(8)]

        # ---------------- dense accumulators ----------------
        NCH = JSH // 512  # 4 chunks of 512 output columns
        dense_ps = [psum_d.tile([B, 512], F32, tag=f"dps{i}", name=f"dps{i}")
                    for i in range(NCH)]

        def dense_fn(t0):
            # matmuls for timesteps t0..t0+15 (h_hist slices 16+t0 ..)
            for dt_ in range(SKIP):
                t_ = t0 + dt_
                wt_tile = wpool.tile([U, JSH], BF16, tag="wt")
                nc.sync.dma_start(wt_tile, d_wt[t_ * U:(t_ + 1) * U, :])
                lhs = h_hist[:, SKIP + t_, :]
                for ch in range(NCH):
                    nc.tensor.matmul(dense_ps[ch], lhs,
                                     wt_tile[:, ch * 512:(ch + 1) * 512],
                                     start=(t_ == 0), stop=False,
                                     skip_group_check=True)

        # ---------------- encoder ----------------
        _lstm_phase(nc, tc, pools, cfg, is_enc=True,
                    k_tiles=enc_k_g, rk_tiles=enc_rk_g, k2_tile=enc_k2,
                    b2_col=enc_b2, bias_row=enc_b4, ones_row=gsel,
                    s0=enc_s0, xc_tile=None, ident=ident, x1t_tile=x1t,
                    c_hist=c_hist, h_hist=None, ifo_ring=ifo_ring,
                    enc_h_chain=enc_h_chain, dense_fn=None)

        re_t = enc_h_chain[8]  # encoder h_127  [U, B] bf16
        if dbg:
            nc.sync.dma_start(d_dbg_ce, c_hist)
            for ci_ in range(8):
                nc.sync.dma_start(d_dbg_he[:, ci_, :], enc_h_chain[ci_ + 1])

        # ---------------- decoder constant x-term ----------------
        # xc = dec_kernel^T @ RE + dec_bias, gate order [i|f|o|cand]
        xc_ps = psum_g.tile([U, 4 * B], F32, tag="gates_ps")
        for g in range(4):
            gg = REF[g]
            gsl = xc_ps[:, g * B:(g + 1) * B]
            nc.tensor.matmul(gsl, dec_k[:, gg * U:(gg + 1) * U], re_t,
                             start=True, stop=False, skip_group_check=True)
            nc.tensor.matmul(gsl, dec_b[:, gg * U:(gg + 1) * U],
                             ones[:, :B], start=False, stop=True,
                             skip_group_check=True)
        xc = consts.tile([U, 4 * B], BF16)
        nc.scalar.copy(xc, xc_ps)

        # ---------------- decoder (+ dense) ----------------
        _lstm_phase(nc, tc, pools, cfg, is_enc=False,
                    k_tiles=None, rk_tiles=dec_rk_g, k2_tile=dec_k2,
                    b2_col=dec_b2, bias_row=None, ones_row=ones,
                    s0=dec_s0, xc_tile=xc, ident=ident, x1t_tile=None,
                    c_hist=c_hist, h_hist=h_hist, ifo_ring=ifo_ring,
                    enc_h_chain=None, dense_fn=dense_fn)

        if dbg:
            nc.sync.dma_start(d_dbg_re, re_t)
            nc.sync.dma_start(d_dbg_c, c_hist)
            nc.sync.dma_start(d_dbg_h, h_hist)
            nc.sync.dma_start(d_dbg_xc, xc)

        # ---------------- dense bias + evacuate + store ----------------
        out_sb = consts.tile([B, JSH], F32)
        for ch in range(NCH):
            # exact fp32 bias add via K=1 matmul: ones^T (x) bias_row
            nc.tensor.matmul(dense_ps[ch], ones[:, :B],
                             db_row[:, ch * 512:(ch + 1) * 512],
                             start=False, stop=True, skip_group_check=True)
            nc.scalar.copy(out_sb[:, ch * 512:(ch + 1) * 512], dense_ps[ch])
        nc.sync.dma_start(d_out, out_sb)

    nc.compile()
    return nc


def _prep_inputs(X, enc_kernel, enc_rkernel, enc_kernel2, enc_bias, enc_bias2,
                 dec_kernel, dec_rkernel, dec_kernel2, dec_bias, dec_bias2,
                 dense_w, dense_b):
    """Host-side sharding/layout prep. Returns in_maps (list of 8 dicts)."""
    x1t = np.ascontiguousarray(
        X[:, 0].transpose(2, 1, 0)).astype(nbf)         # (M, T, B)
    common = {
        "x1t": x1t,
        "enc_k": enc_kernel.astype(nbf),
        "enc_rk": enc_rkernel.astype(nbf),
        "enc_k2": enc_kernel2.astype(nbf),
        "dec_k": dec_kernel.astype(nbf),
        "dec_rk": dec_rkernel.astype(nbf),
        "dec_k2": dec_kernel2.astype(nbf),
        # per-gate bias rows in device gate order [i|f|o|cand] for the K=4
        # gate-select start matmul
        "enc_b4": np.stack([enc_bias[r * U:(r + 1) * U]
                            for r in (0, 1, 3, 2)]).astype(np.float32),
        "gsel": np.repeat(np.eye(4, dtype=np.float32), B, axis=1),
        "dec_b": dec_bias.reshape(1, -1).astype(np.float32),
        "enc_b2": enc_bias2.reshape(-1, 1).astype(np.float32),
        "dec_b2": dec_bias2.reshape(-1, 1).astype(np.float32),
        "ident": np.eye(U, dtype=np.float32).astype(nbf),
        "ones": np.ones((1, 512), np.float32),
    }
    wbf = dense_w.astype(nbf)
    in_maps = []
    for c in range(NCORES):
        m = dict(common)
        m["wt"] = np.ascontiguousarray(wbf[c * JSH:(c + 1) * JSH, :].T)
        m["db"] = dense_b[c * JSH:(c + 1) * JSH].reshape(1, -1).astype(np.float32)
        in_maps.append(m)
    return in_maps


def kernel(X, enc_kernel, enc_rkernel, enc_kernel2, enc_bias, enc_bias2,
           enc_s0, dec_kernel, dec_rkernel, dec_kernel2, dec_bias, dec_bias2,
           dec_s0, dense_w, dense_b, _trace=False):
    cfg = {
        "enc_s0": float(enc_s0),
        "dec_s0": float(dec_s0),
        "enc_bias_nz": bool(np.any(enc_bias != 0)),
        "w_bufs": 22,
    }
    key = tuple(sorted(cfg.items()))
    if key not in _GRAPH_CACHE:
        _GRAPH_CACHE[key] = build_graph(cfg)
    nc = _GRAPH_CACHE[key]

    in_maps = _prep_inputs(
        np.asarray(X), np.asarray(enc_kernel), np.asarray(enc_rkernel),
        np.asarray(enc_kernel2), np.asarray(enc_bias), np.asarray(enc_bias2),
        np.asarray(dec_kernel), np.asarray(dec_rkernel), np.asarray(dec_kernel2),
        np.asarray(dec_bias), np.asarray(dec_bias2),
        np.asarray(dense_w), np.asarray(dense_b))

    res = run_bass_kernel_spmd(nc, in_maps, core_ids=list(range(NCORES)),
                               trace=_trace)
    Y = np.concatenate([res.results[c]["out"] for c in range(NCORES)], axis=1)
    out = Y.reshape(B, T, M).astype(np.float32)
    if _trace:
        return out, res
    return out


if __name__ == "__main__":
    # smoke test with random data
    rng = np.random.default_rng(0)
    s_in = 1.0 / np.sqrt(M)
    s_u = 1.0 / np.sqrt(U)
    s_d = 1.0 / np.sqrt(T * M)
    inputs = {
        "X": rng.standard_normal((B, 2, T, M), dtype=np.float32),
        "enc_kernel": rng.standard_normal((M, 4 * U), dtype=np.float32) * s_in,
        "enc_rkernel": rng.standard_normal((U, 4 * U), dtype=np.float32) * s_u,
        "enc_kernel2": rng.standard_normal((U, U), dtype=np.float32) * s_u,
        "enc_bias": np.zeros(4 * U, np.float32),
        "enc_bias2": np.zeros(U, np.float32),
        "enc_s0": np.float32(0.5),
        "dec_kernel": rng.standard_normal((U, 4 * U), dtype=np.float32) * s_u,
        "dec_rkernel": rng.standard_normal((U, 4 * U), dtype=np.float32) * s_u,
        "dec_kernel2": rng.standard_normal((U, U), dtype=np.float32) * s_u,
        "dec_bias": np.zeros(4 * U, np.float32),
        "dec_bias2": np.zeros(U, np.float32),
        "dec_s0": np.float32(0.5),
        "dense_w": (rng.standard_normal((T * M, T * M), dtype=np.float32) * s_d),
        "dense_b": np.zeros(T * M, np.float32),
    }
    y = kernel(**inputs)
    print("kernel output", y.shape, y.dtype, float(np.abs(y).mean()))


# revision 40
# speedup vs baseline: 1.0123x; 1.0123x over previous
"""Trainium2 Bass kernel for the skip-LSTM autoencoder.

Strategy (8 NeuronCores, zero collectives):
  - Every core runs the full-batch (B=64) encoder + decoder recurrences
    replicated (they are latency-bound; replication costs nothing extra).
  - The 16384x16384 dense layer is column-sharded: core c computes output
    columns [2048c, 2048(c+1)).  The weight shard is pre-transposed on the
    host to (K=16384, J=2048) bf16 so k-tiles stream as perfectly
    contiguous [128, 2048] DMA blocks.
  - All state lives transposed: [U=128 partitions, B=64 free].  Each
    decoder step's hidden tile h_t^T is immediately the dense matmul
    stationary (lhsT); the dense output accumulates in PSUM across all
    128 timesteps and is evacuated once.

Structure facts exploited from the reference recurrence:
  - The i/f/o gate chains are self-contained (gate_t depends only on
    gate_{t-1} and x_t); only the candidate path consumes c_{t-1}.
  - h_t is consumed by nothing except the +16-step skip connection and
    the dense layer, so h is computed in 16-wide batches off the
    critical path.
"""

import sys
from contextlib import ExitStack

sys.path.insert(0, "/opt/trn_rl_repo")

import numpy as np
import ml_dtypes

import concourse.bass as bass
import concourse.mybir as mybir
import concourse.tile as tile
from concourse import bacc
from concourse.bass_utils import run_bass_kernel_spmd

BF16 = mybir.dt.bfloat16
F32 = mybir.dt.float32
AF = mybir.ActivationFunctionType
ALU = mybir.AluOpType

U = 128      # units
B = 64       # batch
T = 128      # sequence length
M = 128      # input feature dim
SKIP = 16
NCORES = 8
JSH = (T * M) // NCORES  # 2048 output columns per core

nbf = ml_dtypes.bfloat16

# module-level cache: (key) -> (nc, meta)
_GRAPH_CACHE = {}


def _lstm_phase(nc, tc, pools, cfg, *, is_enc, k_tiles, rk_tiles, k2_tile,
                b2_col, bias_ifo, gsel_ifo, bias_c, ones_row, s0, xc_tile,
                ident, x1t_tile, c_hist, h_hist, ifo_ring, enc_h_chain,
                dense_fn):
    """Emit one skip-LSTM unroll (128 steps).

    is_enc: encoder (per-step x matmuls from x1t_tile, h only for the
            t%16==15 chain) vs decoder (constant xc via identity matmul,
            h for every step via 16-wide batched tails + per-step dense).

    The candidate-gate pre-activation lives in its own PSUM bank so the
    i/f/o sigmoid (ACT read) never serializes against MM_c (PE write) —
    PSUM PE-write/ACT-read exclusion is bank-granular.
    """
    psum_g = pools["psum_g"]
    psum_c = pools["psum_c"]
    tmp = pools["tmp"]
    tmpb = pools["tmpb"]
    gp_ps = pools["skip_ps"]

    for t in range(T):
        # ---- gate pre-activations: ps_ifo [128, 192], ps_c [128, 64]
        # PSUM gotcha: start=True resets has_written for the WHOLE bank, so
        # exactly one full-width start matmul per bank must come first.
        ps_ifo = psum_g.tile([U, 3 * B], F32, tag="gates_ps")
        ps_c = psum_c.tile([U, B], F32, tag="cand_ps")
        if is_enc:
            # full-width start: bias broadcast via gate-select matmul
            nc.tensor.matmul(ps_ifo, bias_ifo, gsel_ifo, start=True,
                             stop=False)
            nc.tensor.matmul(ps_c, bias_c, ones_row[:, :B], start=True,
                             stop=False)
            # x-term: 4 matmuls kernel_g^T @ x_t
            for g in range(3):
                nc.tensor.matmul(ps_ifo[:, g * B:(g + 1) * B], k_tiles[g],
                                 x1t_tile[:, t, :], start=False, stop=(t == 0),
                                 skip_group_check=True)
            nc.tensor.matmul(ps_c, k_tiles[3], x1t_tile[:, t, :],
                             start=False, stop=(t == 0), skip_group_check=True)
        else:
            # constant x-term (incl. bias) via identity matmuls
            nc.tensor.matmul(ps_ifo, ident, xc_tile[:, 0:3 * B], start=True,
                             stop=(t == 0))
            nc.tensor.matmul(ps_c, ident, xc_tile[:, 3 * B:4 * B], start=True,
                             stop=(t == 0))
        if t > 0:
            prev = ifo_ring[:, (t - 1) % 32, :]
            for g in range(3):
                rk = rk_tiles[g] if g < 2 else rk_tiles[3]
                nc.tensor.matmul(ps_ifo[:, g * B:(g + 1) * B], rk,
                                 prev[:, g * B:(g + 1) * B],
                                 start=False, stop=True, skip_group_check=True)
            nc.tensor.matmul(ps_c, rk_tiles[2], c_hist[:, t - 1, :],
                             start=False, stop=True, skip_group_check=True)

        # ---- activations: one sigmoid over [i,f,o], one tanh for cand
        ifo = ifo_ring[:, t % 32, :]
        nc.scalar.activation(ifo, ps_ifo, AF.Sigmoid)
        u_t = tmp.tile([U, B], BF16, tag="u")
        nc.scalar.activation(u_t, ps_c, AF.Tanh)

        # ---- cell update: c_t = f*c_{t-1} + i*u   (bf16 state)
        if t == 0:
            nc.vector.tensor_tensor(c_hist[:, 0, :], ifo[:, 0:B], u_t,
                                    ALU.mult)
        else:
            m1 = tmp.tile([U, B], BF16, tag="m1")
            nc.vector.tensor_tensor(m1, ifo[:, 0:B], u_t, ALU.mult)
            fb = tmp.tile([U, B], BF16, tag="fb")
            nc.gpsimd.tensor_tensor(fb, ifo[:, B:2 * B], c_hist[:, t - 1, :],
                                    ALU.mult)
            nc.vector.tensor_tensor(c_hist[:, t, :], m1, fb, ALU.add)

        # ---- h tail
        if is_enc:
            if t % SKIP == SKIP - 1:
                ci = t // SKIP          # chain index 0..7
                h_prev = enc_h_chain[ci]  # holds h_{t-16} (ci==0: zeros)
                sps = gp_ps.tile([U, SKIP * B], F32, tag="skip_ps")
                nc.tensor.matmul(sps[:, 0:B], k2_tile, h_prev,
                                 start=True, stop=True, skip_group_check=True)
                skp = tmp.tile([U, B], BF16, tag="skp")
                nc.scalar.activation(skp, sps[:, 0:B], AF.Sigmoid, bias=b2_col)
                tc_t = tmp.tile([U, B], BF16, tag="tc")
                nc.scalar.activation(tc_t, c_hist[:, t, :], AF.Tanh)
                hc = tmp.tile([U, B], BF16, tag="hc")
                nc.vector.tensor_tensor(hc, ifo[:, 2 * B:3 * B], tc_t, ALU.mult)
                skp2 = tmp.tile([U, B], BF16, tag="skp2")
                nc.gpsimd.tensor_scalar_mul(skp2, skp, 1.0 - s0)
                nc.vector.scalar_tensor_tensor(
                    enc_h_chain[ci + 1], hc, s0, skp2, ALU.mult, ALU.add)
        else:
            if t % SKIP == SKIP - 1:
                t0 = t - (SKIP - 1)
                W = SKIP * B  # 1024
                # skip pre-act for steps t0..t0+15: inputs h_{t0-16..t0-1}
                # = h_hist slices [t0 .. t0+16)  (h_hist has 16 zero slices
                # in front).  Two N=512 matmuls: PSUM bank limit.
                sps = gp_ps.tile([U, W], F32, tag="skip_ps")
                nc.tensor.matmul(sps[:, 0:W // 2], k2_tile,
                                 h_hist[:, t0:t0 + SKIP // 2, :],
                                 start=True, stop=True, skip_group_check=True)
                nc.tensor.matmul(sps[:, W // 2:W], k2_tile,
                                 h_hist[:, t0 + SKIP // 2:t0 + SKIP, :],
                                 start=True, stop=True, skip_group_check=True)
                skp = tmpb.tile([U, W], BF16, tag="skp_b")
                nc.scalar.activation(skp, sps, AF.Sigmoid, bias=b2_col)
                tc_b = tmpb.tile([U, W], BF16, tag="tc_b")
                nc.scalar.activation(
                    tc_b,
                    c_hist[:, t0:t0 + SKIP, :].rearrange("p s b -> p (s b)"),
                    AF.Tanh)
                hc = tmpb.tile([U, W], BF16, tag="hc_b")
                nc.vector.tensor_tensor(
                    hc.rearrange("p (s b) -> p s b", s=SKIP),
                    ifo_ring[:, t0 % 32:t0 % 32 + SKIP, 2 * B:3 * B],
                    tc_b.rearrange("p (s b) -> p s b", s=SKIP), ALU.mult)
                skp2 = tmpb.tile([U, W], BF16, tag="skp2_b")
                nc.gpsimd.tensor_scalar_mul(skp2, skp, 1.0 - s0)
                nc.vector.scalar_tensor_tensor(
                    h_hist[:, SKIP + t0:SKIP + t0 + SKIP, :].rearrange(
                        "p s b -> p (s b)"),
                    hc, s0, skp2, ALU.mult, ALU.add)
            # dense matmuls, one lagged timestep per step: h(t-16) is
            # materialized by its group's batched tail at step t-1 or
            # earlier, so it never stalls and the weight stream spreads
            # evenly across the phase.
            if dense_fn is not None and t >= SKIP:
                dense_fn(t - SKIP)


def build_graph(cfg):
    """Build the SPMD graph (identical on all cores)."""
    nc = bacc.Bacc("TRN2", target_bir_lowering=False, debug=False,
                   num_devices=NCORES)

    # ---------------- DRAM parameters ----------------
    d_x1t = nc.dram_tensor("x1t", [M, T, B], BF16, kind="ExternalInput").ap()
    d_enc_k = nc.dram_tensor("enc_k", [M, 4 * U], BF16, kind="ExternalInput").ap()
    d_enc_rk = nc.dram_tensor("enc_rk", [U, 4 * U], BF16, kind="ExternalInput").ap()
    d_enc_k2 = nc.dram_tensor("enc_k2", [U, U], BF16, kind="ExternalInput").ap()
    d_dec_k = nc.dram_tensor("dec_k", [U, 4 * U], BF16, kind="ExternalInput").ap()
    d_dec_rk = nc.dram_tensor("dec_rk", [U, 4 * U], BF16, kind="ExternalInput").ap()
    d_dec_k2 = nc.dram_tensor("dec_k2", [U, U], BF16, kind="ExternalInput").ap()
    d_enc_b4 = nc.dram_tensor("enc_b4", [4, U], F32, kind="ExternalInput").ap()
    d_enc_bc = nc.dram_tensor("enc_bc", [1, U], F32, kind="ExternalInput").ap()
    d_gsel = nc.dram_tensor("gsel", [4, 4 * B], F32, kind="ExternalInput").ap()
    d_dec_b = nc.dram_tensor("dec_b", [1, 4 * U], F32, kind="ExternalInput").ap()
    d_enc_b2 = nc.dram_tensor("enc_b2", [U, 1], F32, kind="ExternalInput").ap()
    d_dec_b2 = nc.dram_tensor("dec_b2", [U, 1], F32, kind="ExternalInput").ap()
    d_ident = nc.dram_tensor("ident", [U, U], BF16, kind="ExternalInput").ap()
    d_ones = nc.dram_tensor("ones", [1, 512], F32, kind="ExternalInput").ap()
    d_wt = nc.dram_tensor("wt", [T * M, JSH], BF16, kind="ExternalInput").ap()
    d_db = nc.dram_tensor("db", [1, JSH], F32, kind="ExternalInput").ap()
    d_out = nc.dram_tensor("out", [B, JSH], F32, kind="ExternalOutput").ap()
    dbg = cfg.get("debug", False)
    if dbg:
        pass  # debug tensors declared below; APs stashed into cfg
    if dbg:
        d_dbg_re = nc.dram_tensor("dbg_re", [U, B], BF16,
                                  kind="ExternalOutput").ap()
        d_dbg_c = nc.dram_tensor("dbg_c", [U, T, B], BF16,
                                 kind="ExternalOutput").ap()
        d_dbg_h = nc.dram_tensor("dbg_h", [U, SKIP + T, B], BF16,
                                 kind="ExternalOutput").ap()
        d_dbg_xc = nc.dram_tensor("dbg_xc", [U, 4 * B], BF16,
                                  kind="ExternalOutput").ap()
        d_dbg_ce = nc.dram_tensor("dbg_ce", [U, T, B], BF16,
                                  kind="ExternalOutput").ap()
        d_dbg_he = nc.dram_tensor("dbg_he", [U, 8, B], BF16,
                                  kind="ExternalOutput").ap()
        d_dbg_ps1 = nc.dram_tensor("dbg_ps1", [U, 4 * B], F32,
                                   kind="ExternalOutput").ap()
        d_dbg_ifo0 = nc.dram_tensor("dbg_ifo0", [U, 3 * B], BF16,
                                    kind="ExternalOutput").ap()
        cfg["d_dbg_ps1"] = d_dbg_ps1
        cfg["d_dbg_ifo0"] = d_dbg_ifo0

    enc_s0 = cfg["enc_s0"]
    dec_s0 = cfg["dec_s0"]

    with tile.TileContext(nc) as tc, ExitStack() as ctx:
        consts = ctx.enter_context(tc.tile_pool(name="consts", bufs=1))
        hist = ctx.enter_context(tc.tile_pool(name="hist", bufs=1))
        tmp = ctx.enter_context(tc.tile_pool(name="tmp", bufs=4))
        tmpb = ctx.enter_context(tc.tile_pool(name="tmpb", bufs=2))
        wpool = ctx.enter_context(tc.tile_pool(name="wpool", bufs=cfg["w_bufs"]))
        psum_g = ctx.enter_context(tc.tile_pool(name="psum_g", bufs=2, space="PSUM"))
        psum_c = ctx.enter_context(tc.tile_pool(name="psum_c", bufs=2, space="PSUM"))
        skip_ps = ctx.enter_context(tc.tile_pool(name="skip_ps", bufs=1, space="PSUM"))
        psum_d = ctx.enter_context(tc.tile_pool(name="psum_d", bufs=1, space="PSUM"))

        pools = {"psum_g": psum_g, "psum_c": psum_c, "tmp": tmp,
                 "tmpb": tmpb, "skip_ps": skip_ps}

        # ---------------- load constants ----------------
        x1t = consts.tile([M, T, B], BF16)
        nc.sync.dma_start(x1t, d_x1t)

        def load_w(dram, shape, nm):
            t_ = consts.tile(shape, BF16, tag=nm, name=nm)
            nc.sync.dma_start(t_, dram)
            return t_

        enc_k = load_w(d_enc_k, [M, 4 * U], "enc_k_t")
        enc_rk = load_w(d_enc_rk, [U, 4 * U], "enc_rk_t")
        enc_k2 = load_w(d_enc_k2, [U, U], "enc_k2_t")
        dec_k = load_w(d_dec_k, [U, 4 * U], "dec_k_t")
        dec_rk = load_w(d_dec_rk, [U, 4 * U], "dec_rk_t")
        dec_k2 = load_w(d_dec_k2, [U, U], "dec_k2_t")
        ident = load_w(d_ident, [U, U], "ident_t")

        enc_b2 = consts.tile([U, 1], F32)
        nc.sync.dma_start(enc_b2, d_enc_b2)
        dec_b2 = consts.tile([U, 1], F32)
        nc.sync.dma_start(dec_b2, d_dec_b2)
        ones = consts.tile([1, 512], F32)
        nc.sync.dma_start(ones, d_ones)
        enc_b4 = consts.tile([4, U], F32)
        nc.sync.dma_start(enc_b4, d_enc_b4)
        enc_bc = consts.tile([1, U], F32)
        nc.sync.dma_start(enc_bc, d_enc_bc)
        gsel = consts.tile([4, 4 * B], F32)
        nc.sync.dma_start(gsel, d_gsel)
        dec_b = consts.tile([1, 4 * U], F32)
        nc.sync.dma_start(dec_b, d_dec_b)
        db_row = consts.tile([1, JSH], F32)
        nc.sync.dma_start(db_row, d_db)

        # gate column order in psum: [i | f | o | cand]; reference weight
        # column order is [i | f | cand | o].  Map: psum gate 0->ref 0,
        # 1->ref 1, 2->ref 3, 3->ref 2.
        REF = [0, 1, 3, 2]
        enc_k_g = [enc_k[:, REF[g] * U:(REF[g] + 1) * U] for g in range(4)]
        enc_rk_g = [enc_rk[:, gg * U:(gg + 1) * U] for gg in range(4)]
        dec_rk_g = [dec_rk[:, gg * U:(gg + 1) * U] for gg in range(4)]

        # ---------------- state buffers ----------------
        c_hist = hist.tile([U, T, B], BF16)       # cell state history
        h_hist = hist.tile([U, SKIP + T, B], BF16)  # decoder h (16 zero pad)
        ifo_ring = hist.tile([U, 32, 3 * B], BF16)
        nc.vector.memset(h_hist[:, 0:SKIP, :], 0.0)

        zeros_h = consts.tile([U, B], BF16)
        nc.vector.memset(zeros_h, 0.0)
        enc_h_chain = [zeros_h] + [
            hist.tile([U, B], BF16, tag=f"ehc{i}", name=f"ehc{i}")
            for i in range(8)]

        # ---------------- dense accumulators ----------------
        # 4 chunks of 512 output columns, packed two per PSUM bank: chunk
        # 2i -> partitions 0:64, chunk 2i+1 -> partitions 64:128 via PE
        # column-group tiling (the two matmuls stream concurrently).
        NCH = JSH // 512
        dense_ps = [psum_d.tile([2 * B, 512], F32, tag=f"dps{i}",
                                name=f"dps{i}") for i in range(NCH // 2)]

        def dense_fn(t_):
            # dense matmuls for one timestep (h_hist slice 16+t_)
            wt_tile = wpool.tile([U, JSH], BF16, tag="wt")
            nc.sync.dma_start(wt_tile, d_wt[t_ * U:(t_ + 1) * U, :])
            lhs = h_hist[:, SKIP + t_, :]
            for ch in range(NCH):
                out = dense_ps[ch // 2][(ch % 2) * B:(ch % 2 + 1) * B, :]
                nc.tensor.matmul(out, lhs,
                                 wt_tile[:, ch * 512:(ch + 1) * 512],
                                 start=(t_ == 0), stop=False,
                                 skip_group_check=True,
                                 tile_position=(0, (ch % 2) * B))

        # ---------------- encoder ----------------
        _lstm_phase(nc, tc, pools, cfg, is_enc=True,
                    k_tiles=enc_k_g, rk_tiles=enc_rk_g, k2_tile=enc_k2,
                    b2_col=enc_b2, bias_ifo=enc_b4[0:3, :],
                    gsel_ifo=gsel[0:3, 0:3 * B], bias_c=enc_bc,
                    ones_row=ones,
                    s0=enc_s0, xc_tile=None, ident=ident, x1t_tile=x1t,
                    c_hist=c_hist, h_hist=None, ifo_ring=ifo_ring,
                    enc_h_chain=enc_h_chain, dense_fn=None)

        re_t = enc_h_chain[8]  # encoder h_127  [U, B] bf16
        if dbg:
            nc.sync.dma_start(d_dbg_ce, c_hist)
            for ci_ in range(8):
                nc.sync.dma_start(d_dbg_he[:, ci_, :], enc_h_chain[ci_ + 1])

        # ---------------- decoder constant x-term ----------------
        # xc = dec_kernel^T @ RE + dec_bias, gate order [i|f|o|cand]
        xc_big = skip_ps.tile([U, SKIP * B], F32, tag="skip_ps",
                              name="xc_big")
        xc_ps = xc_big[:, 0:4 * B]
        for g in range(4):
            gg = REF[g]
            gsl = xc_ps[:, g * B:(g + 1) * B]
            nc.tensor.matmul(gsl, dec_k[:, gg * U:(gg + 1) * U], re_t,
                             start=True, stop=False, skip_group_check=True)
            nc.tensor.matmul(gsl, dec_b[:, gg * U:(gg + 1) * U],
                             ones[:, :B], start=False, stop=True,
                             skip_group_check=True)
        xc = consts.tile([U, 4 * B], BF16)
        nc.scalar.copy(xc, xc_ps)

        # ---------------- decoder (+ dense) ----------------
        _lstm_phase(nc, tc, pools, cfg, is_enc=False,
                    k_tiles=None, rk_tiles=dec_rk_g, k2_tile=dec_k2,
                    b2_col=dec_b2, bias_ifo=None, gsel_ifo=None, bias_c=None,
                    ones_row=ones,
                    s0=dec_s0, xc_tile=xc, ident=ident, x1t_tile=None,
                    c_hist=c_hist, h_hist=h_hist, ifo_ring=ifo_ring,
                    enc_h_chain=None, dense_fn=dense_fn)
        # dense epilogue: last 16 timesteps
        for t_ in range(T - SKIP, T):
            dense_fn(t_)

        if dbg:
            nc.sync.dma_start(d_dbg_re, re_t)
            nc.sync.dma_start(d_dbg_c, c_hist)
            nc.sync.dma_start(d_dbg_h, h_hist)
            nc.sync.dma_start(d_dbg_xc, xc)

        # ---------------- dense bias + evacuate + store ----------------
        # chunk ch output sits at psum tile ch//2, partitions (ch%2)*64..
        out_sb = consts.tile([2 * B, 2 * 512], F32)
        for ch in range(NCH):
            pslice = dense_ps[ch // 2][(ch % 2) * B:(ch % 2 + 1) * B, :]
            # exact fp32 bias add via K=1 matmul: ones^T (x) bias_row
            nc.tensor.matmul(pslice, ones[:, :B],
                             db_row[:, ch * 512:(ch + 1) * 512],
                             start=False, stop=True, skip_group_check=True,
                             tile_position=(0, (ch % 2) * B))
            osl = out_sb[(ch % 2) * B:(ch % 2 + 1) * B,
                         (ch // 2) * 512:(ch // 2 + 1) * 512]
            nc.scalar.copy(osl, pslice)
            nc.sync.dma_start(d_out[:, ch * 512:(ch + 1) * 512], osl)

    nc.compile()
    return nc


def _prep_inputs(X, enc_kernel, enc_rkernel, enc_kernel2, enc_bias, enc_bias2,
                 dec_kernel, dec_rkernel, dec_kernel2, dec_bias, dec_bias2,
                 dense_w, dense_b):
    """Host-side sharding/layout prep. Returns in_maps (list of 8 dicts)."""
    x1t = np.ascontiguousarray(
        X[:, 0].transpose(2, 1, 0)).astype(nbf)         # (M, T, B)
    common = {
        "x1t": x1t,
        "enc_k": enc_kernel.astype(nbf),
        "enc_rk": enc_rkernel.astype(nbf),
        "enc_k2": enc_kernel2.astype(nbf),
        "dec_k": dec_kernel.astype(nbf),
        "dec_rk": dec_rkernel.astype(nbf),
        "dec_k2": dec_kernel2.astype(nbf),
        # per-gate bias rows in device gate order [i|f|o|cand] for the K=4
        # gate-select start matmul
        "enc_b4": np.stack([enc_bias[r * U:(r + 1) * U]
                            for r in (0, 1, 3, 2)]).astype(np.float32),
        "gsel": np.repeat(np.eye(4, dtype=np.float32), B, axis=1),
        "enc_bc": enc_bias[2 * U:3 * U].reshape(1, -1).astype(np.float32),
        "dec_b": dec_bias.reshape(1, -1).astype(np.float32),
        "enc_b2": enc_bias2.reshape(-1, 1).astype(np.float32),
        "dec_b2": dec_bias2.reshape(-1, 1).astype(np.float32),
        "ident": np.eye(U, dtype=np.float32).astype(nbf),
        "ones": np.ones((1, 512), np.float32),
    }
    wbf = dense_w.astype(nbf)
    in_maps = []
    for c in range(NCORES):
        m = dict(common)
        m["wt"] = np.ascontiguousarray(wbf[c * JSH:(c + 1) * JSH, :].T)
        m["db"] = dense_b[c * JSH:(c + 1) * JSH].reshape(1, -1).astype(np.float32)
        in_maps.append(m)
    return in_maps


def kernel(X, enc_kernel, enc_rkernel, enc_kernel2, enc_bias, enc_bias2,
           enc_s0, dec_kernel, dec_rkernel, dec_kernel2, dec_bias, dec_bias2,
           dec_s0, dense_w, dense_b, _trace=False):
    cfg = {
        "enc_s0": float(enc_s0),
        "dec_s0": float(dec_s0),
        "enc_bias_nz": bool(np.any(enc_bias != 0)),
        "w_bufs": 22,
    }
    key = tuple(sorted(cfg.items()))
    if key not in _GRAPH_CACHE:
        _GRAPH_CACHE[key] = build_graph(cfg)
    nc = _GRAPH_CACHE[key]

    in_maps = _prep_inputs(
        np.asarray(X), np.asarray(enc_kernel), np.asarray(enc_rkernel),
        np.asarray(enc_kernel2), np.asarray(enc_bias), np.asarray(enc_bias2),
        np.asarray(dec_kernel), np.asarray(dec_rkernel), np.asarray(dec_kernel2),
        np.asarray(dec_bias), np.asarray(dec_bias2),
        np.asarray(dense_w), np.asarray(dense_b))

    res = run_bass_kernel_spmd(nc, in_maps, core_ids=list(range(NCORES)),
                               trace=_trace)
    Y = np.concatenate([res.results[c]["out"] for c in range(NCORES)], axis=1)
    out = Y.reshape(B, T, M).astype(np.float32)
    if _trace:
        return out, res
    return out


if __name__ == "__main__":
    # smoke test with random data
    rng = np.random.default_rng(0)
    s_in = 1.0 / np.sqrt(M)
    s_u = 1.0 / np.sqrt(U)
    s_d = 1.0 / np.sqrt(T * M)
    inputs = {
        "X": rng.standard_normal((B, 2, T, M), dtype=np.float32),
        "enc_kernel": rng.standard_normal((M, 4 * U), dtype=np.float32) * s_in,
        "enc_rkernel": rng.standard_normal((U, 4 * U), dtype=np.float32) * s_u,
        "enc_kernel2": rng.standard_normal((U, U), dtype=np.float32) * s_u,
        "enc_bias": np.zeros(4 * U, np.float32),
        "enc_bias2": np.zeros(U, np.float32),
        "enc_s0": np.float32(0.5),
        "dec_kernel": rng.standard_normal((U, 4 * U), dtype=np.float32) * s_u,
        "dec_rkernel": rng.standard_normal((U, 4 * U), dtype=np.float32) * s_u,
        "dec_kernel2": rng.standard_normal((U, U), dtype=np.float32) * s_u,
        "dec_bias": np.zeros(4 * U, np.float32),
        "dec_bias2": np.zeros(U, np.float32),
        "dec_s0": np.float32(0.5),
        "dense_w": (rng.standard_normal((T * M, T * M), dtype=np.float32) * s_d),
        "dense_b": np.zeros(T * M, np.float32),
    }
    y = kernel(**inputs)
    print("kernel output", y.shape, y.dtype, float(np.abs(y).mean()))
